# revision 8
# baseline (speedup 1.0000x reference)
"""Multi-head attention (single query vector per batch) on 8 TRN2 NeuronCores.

Problem: nn_MultiHeadAttention  (B=32, T=4096, D=A=V=1024, H=16 heads)

    q = vector @ Wq + bq                  # [B, A]
    k = matrix @ Wk + bk                  # [B, T, A]
    v = matrix @ Wv + bv                  # [B, T, V]
    scores[b,h,t] = (q_h/8) . k_h         # [B, H, T]
    attn = softmax(mask ? scores : -1e30)
    out = (attn @ v_h) @ Wo + bo          # [B, D]

Key algebra: since there is ONE query per batch, k never needs to be
materialized.  scores[b,h,t] = matrix[b,t,:] . wq_eff[b,h,:] + cb[b,h]
with wq_eff[b,h,c] = sum_d Wk[c, h*64+d] * qs[b, h*64+d]  (qs = (q+bq)/8)
and cb[b,h] = sum_d bk[h*64+d] * qs[b,h*64+d].  Likewise the attention
output is (attn @ matrix) @ Wv_head + bv, so v is never materialized.
This turns a 550-GFLOP problem into a ~9-GFLOP memory-bound stream over
`matrix`.

Sharding: data-parallel over batch, 4 batches per core, no collectives.
Each core streams its matrix shard twice from HBM: once channel-major
(host-pre-transposed) feeding the scores matmul (contract over channels)
and once natural feeding the context matmul (contract over T).  Softmax
normalization is deferred: p = exp(s + maskneg) accumulates unnormalized
into ctx via PSUM, both scaled by 1/sum(p) at the end of each batch.
"""

import os

import numpy as np

import concourse.bass as bass
import concourse.tile as tile
from concourse import bacc, mybir
from concourse.bass_utils import run_bass_kernel_spmd
from concourse.masks import make_identity

F32 = mybir.dt.float32
F32R = mybir.dt.float32r

B, T, D, H = 32, 4096, 1024, 16
NCORES = 8
BPC = B // NCORES          # batches per core
SCALE = 1.0 / 8.0          # 1 / sqrt(D // H)
NEG = np.float32(-1e30)
P = 128                    # SBUF partitions
CH = D // P                # channel chunks (8)
TBLK = 512                 # t elements per streaming block
NSUB = TBLK // P           # 128-row subtiles per block (4)
HPC = 2                    # heads per 128-channel chunk (128/64)


def ts(i, n):
    return slice(i * n, (i + 1) * n)


def build(bpc=BPC, t=T, use_f32r=False):
    """Build the per-core Bass program (SPMD: same program, 8 cores)."""
    d, h = D, H
    TB = t // TBLK                 # streaming blocks per batch
    BH = bpc * h

    nc = bacc.Bacc("TRN2", target_bir_lowering=False, debug=False)

    mT_d = nc.dram_tensor("mT", [bpc, d, t], F32, kind="ExternalInput")
    mN_d = nc.dram_tensor("mN", [bpc, t, d], F32, kind="ExternalInput")
    vecT_d = nc.dram_tensor("vecT", [d, bpc], F32, kind="ExternalInput")
    mneg_d = nc.dram_tensor("mneg", [bpc, t], F32, kind="ExternalInput")
    wq_d = nc.dram_tensor("Wq", [d, d], F32, kind="ExternalInput")
    wkT_d = nc.dram_tensor("WkT", [d, d], F32, kind="ExternalInput")
    wv_d = nc.dram_tensor("Wv", [d, d], F32, kind="ExternalInput")
    wo_d = nc.dram_tensor("Wo", [d, d], F32, kind="ExternalInput")
    bq_d = nc.dram_tensor("bq", [d], F32, kind="ExternalInput")
    bk_d = nc.dram_tensor("bk", [d], F32, kind="ExternalInput")
    bv_d = nc.dram_tensor("bv", [d], F32, kind="ExternalInput")
    bo_d = nc.dram_tensor("bo", [d], F32, kind="ExternalInput")

    sco_d = nc.dram_tensor("scores_o", [bpc, h, t], F32, kind="ExternalOutput")
    att_d = nc.dram_tensor("attn_o", [bpc, h, t], F32, kind="ExternalOutput")
    out_d = nc.dram_tensor("out_o", [bpc, d], F32, kind="ExternalOutput")

    def r(ap):  # dtype for the two big streaming matmuls
        return ap.bitcast(F32R) if use_f32r else ap

    def bcast_dram(handle, n_part, free_ap, offset=0):
        """AP reading DRAM replicated across n_part partitions."""
        return bass.AP(
            tensor=handle[:].tensor,
            offset=offset,
            ap=[[0, n_part]] + free_ap,
        )

    with tile.TileContext(nc) as tc:
        with (
            tc.tile_pool(name="wpool", bufs=1) as wpool,
            tc.tile_pool(name="persist", bufs=1) as persist,
            tc.tile_pool(name="pbuf", bufs=2) as pbuf,
            tc.tile_pool(name="stream", bufs=2) as stream,
            tc.tile_pool(name="small", bufs=2) as small,
            tc.tile_pool(name="scratch", bufs=1) as scratch,
            tc.tile_pool(name="ps_s", bufs=2, space="PSUM") as ps_s,
            tc.tile_pool(name="ps_ctx", bufs=1, space="PSUM") as ps_ctx,
            tc.tile_pool(name="ps_t", bufs=2, space="PSUM") as ps_t,
            tc.tile_pool(name="ps_epi", bufs=1, space="PSUM") as ps_epi,
        ):
            # ---------------- setup: identities, replicated biases -------
            ident = persist.tile([16, 16], F32, tag="ident")
            make_identity(nc, ident)

            vecT_sb = persist.tile([P, CH, bpc], F32, tag="vecT")
            nc.sync.dma_start(
                out=vecT_sb, in_=vecT_d[:].rearrange("(o p) b -> p o b", p=P)
            )
            bq_rep = persist.tile([bpc, d], F32, tag="bq_rep")
            nc.gpsimd.dma_start(out=bq_rep, in_=bcast_dram(bq_d, bpc, [[1, d]]))
            bk_rep = persist.tile([bpc, d], F32, tag="bk_rep")
            nc.gpsimd.dma_start(out=bk_rep, in_=bcast_dram(bk_d, bpc, [[1, d]]))
            bo_rep = persist.tile([bpc, d], F32, tag="bo_rep")
            nc.gpsimd.dma_start(out=bo_rep, in_=bcast_dram(bo_d, bpc, [[1, d]]))
            bv_sb = persist.tile([P, CH], F32, tag="bv_sb")
            nc.sync.dma_start(out=bv_sb, in_=bv_d[:].rearrange("(o p) -> p o", p=P))

            # ---------------- q = vector @ Wq + bq;  qs = q/8 ------------
            wq_sb_w = wpool.tile([P, CH, d], F32, tag="w")
            nc.sync.dma_start(
                out=wq_sb_w, in_=wq_d[:].rearrange("(o p) a -> p o a", p=P)
            )
            q_ps = ps_epi.tile([bpc, d], F32, tag="epi")
            for kc in range(CH):
                for nh in range(2):
                    nc.tensor.matmul(
                        q_ps[:, ts(nh, 512)],
                        vecT_sb[:, kc, :],
                        wq_sb_w[:, kc, ts(nh, 512)],
                        start=(kc == 0),
                        stop=(kc == CH - 1),
                    )
            qs_sb = persist.tile([bpc, d], F32, tag="qs")
            nc.vector.tensor_tensor(qs_sb, q_ps, bq_rep, mybir.AluOpType.add)
            nc.vector.tensor_scalar_mul(qs_sb, qs_sb, SCALE)

            # qsT[c, b] via PE transposes of [bpc, 128] chunks
            qsT_sb = persist.tile([P, CH, bpc], F32, tag="qsT")
            for kc in range(CH):
                tp = ps_t.tile([P, NSUB, 16], F32, tag="tps")
                nc.tensor.transpose(
                    tp[:, 0, :bpc], qs_sb[:, ts(kc, P)], ident[:bpc, :bpc]
                )
                nc.any.tensor_copy(out=qsT_sb[:, kc, :], in_=tp[:, 0, :bpc])

            # qs_embT[A, b*16+h] = qs[b, A] if head(A)==h else 0
            qs_embT = persist.tile([P, CH, BH], F32, tag="qs_embT")
            nc.vector.memset(qs_embT, 0.0)
            for kc in range(CH):
                for half in range(HPC):
                    hh = HPC * kc + half
                    dst = qs_embT[ts(half, 64), kc, :].rearrange(
                        "p (b g) -> p b g", g=h
                    )[:, :, hh]
                    nc.any.tensor_copy(out=dst, in_=qsT_sb[ts(half, 64), kc, :])

            # cb[b,h] = sum_d bk[h*64+d]*qs[b,h*64+d];  cbT[h,b]
            cb_tmp = scratch.tile([bpc, d], F32, tag="cb_tmp")
            nc.vector.tensor_tensor(cb_tmp, qs_sb, bk_rep, mybir.AluOpType.mult)
            cb_b = scratch.tile([bpc, h], F32, tag="cb_b")
            nc.vector.tensor_reduce(
                cb_b,
                cb_tmp.rearrange("b (g e) -> b g e", e=d // h),
                axis=mybir.AxisListType.X,
                op=mybir.AluOpType.add,
            )
            cbT_sb = persist.tile([h, bpc], F32, tag="cbT")
            tp = ps_t.tile([P, NSUB, 16], F32, tag="tps")
            nc.tensor.transpose(tp[:h, 0, :bpc], cb_b, ident[:bpc, :bpc])
            nc.any.tensor_copy(out=cbT_sb, in_=tp[:h, 0, :bpc])

            # ------------- wq_eff^T[c, b*16+h] via WkT ------------------
            wkT_sb = wpool.tile([P, CH, d], F32, tag="w")
            nc.sync.dma_start(
                out=wkT_sb, in_=wkT_d[:].rearrange("(o p) c -> p o c", p=P)
            )
            wq_eff = persist.tile([P, CH, BH], F32, tag="wq_eff")
            for ct in range(CH):
                wq_ps = ps_epi.tile([P, BH], F32, tag="epi")
                for ka in range(CH):
                    nc.tensor.matmul(
                        wq_ps,
                        wkT_sb[:, ka, ts(ct, P)],
                        qs_embT[:, ka, :],
                        start=(ka == 0),
                        stop=(ka == CH - 1),
                    )
                nc.any.tensor_copy(out=wq_eff[:, ct, :], in_=wq_ps)

            # ---------------- streaming pass over matrix ----------------
            ctxT_sb = persist.tile([P, CH, bpc, h], F32, tag="ctxT")
            recip_sb = persist.tile([h, bpc], F32, tag="recip")

            for b in range(bpc):
                p_all = pbuf.tile([h, t], F32, tag="p_all")
                partials = pbuf.tile([h, TB], F32, tag="partials")
                ctx_ps = ps_ctx.tile([h, d], F32, tag="ctx")

                for jt in range(TB):
                    mT_t = stream.tile([P, CH, TBLK], F32, tag="mT_t")
                    nc.sync.dma_start(
                        out=mT_t,
                        in_=mT_d[b, :, ts(jt, TBLK)].rearrange(
                            "(o p) t -> p o t", p=P
                        ),
                    )
                    mN_t = stream.tile([P, NSUB, d], F32, tag="mN_t")
                    nc.sync.dma_start(
                        out=mN_t,
                        in_=mN_d[b, ts(jt, TBLK), :].rearrange(
                            "(i p) c -> p i c", p=P
                        ),
                    )
                    mneg_blk = small.tile([h, TBLK], F32, tag="mneg_blk")
                    nc.gpsimd.dma_start(
                        out=mneg_blk,
                        in_=bcast_dram(
                            mneg_d, h, [[1, TBLK]], offset=b * t + jt * TBLK
                        ),
                    )

                    # scores[h, tau] = sum_c wq_eff[c, bh] * mT[c, tau]
                    s_ps = ps_s.tile([h, TBLK], F32, tag="s_ps")
                    for kc in range(CH):
                        nc.tensor.matmul(
                            s_ps,
                            r(wq_eff[:, kc, ts(b, h)]),
                            r(mT_t[:, kc, :]),
                            start=(kc == 0),
                            stop=(kc == CH - 1),
                        )

                    # raw scores out (+cb), masked exp (+cb) and row-sums
                    s_sb = small.tile([h, TBLK], F32, tag="s_sb")
                    nc.vector.tensor_scalar_add(s_sb, s_ps, cbT_sb[:, ts(b, 1)])
                    nc.sync.dma_start(out=sco_d[b, :, ts(jt, TBLK)], in_=s_sb)

                    sm_sb = small.tile([h, TBLK], F32, tag="sm_sb")
                    nc.vector.tensor_tensor(
                        sm_sb, s_ps, mneg_blk, mybir.AluOpType.add
                    )
                    nc.scalar.activation(
                        out=p_all[:, ts(jt, TBLK)],
                        in_=sm_sb,
                        func=mybir.ActivationFunctionType.Exp,
                        bias=cbT_sb[:, ts(b, 1)],
                        scale=1.0,
                        accum_out=partials[:, ts(jt, 1)],
                    )

                    # p^T tiles and ctx += p^T.T-style accumulation
                    tp = ps_t.tile([P, NSUB, 16], F32, tag="tps")
                    pT_sb = small.tile([P, NSUB, h], F32, tag="pT_sb")
                    for i in range(NSUB):
                        nc.tensor.transpose(
                            tp[:, i, :h],
                            p_all[:, jt * TBLK + i * P : jt * TBLK + (i + 1) * P],
                            ident[:h, :h],
                        )
                        nc.any.tensor_copy(out=pT_sb[:, i, :], in_=tp[:, i, :h])
                    for i in range(NSUB):
                        for nh in range(2):
                            nc.tensor.matmul(
                                ctx_ps[:, ts(nh, 512)],
                                r(pT_sb[:, i, :]),
                                r(mN_t[:, i, ts(nh, 512)]),
                                start=(jt == 0 and i == 0),
                                stop=(jt == TB - 1 and i == NSUB - 1),
                                skip_group_check=True,
                            )

                # ---- batch epilogue: normalize attn and ctx ----
                tot = small.tile([h, 1], F32, tag="tot")
                nc.vector.tensor_reduce(
                    tot, partials, axis=mybir.AxisListType.X, op=mybir.AluOpType.add
                )
                nc.vector.reciprocal(recip_sb[:, ts(b, 1)], tot)
                for jt in range(TB):
                    att_stg = small.tile([h, TBLK], F32, tag="att_stg")
                    nc.scalar.activation(
                        out=att_stg,
                        in_=p_all[:, ts(jt, TBLK)],
                        func=mybir.ActivationFunctionType.Copy,
                        scale=recip_sb[:, ts(b, 1)],
                    )
                    nc.sync.dma_start(out=att_d[b, :, ts(jt, TBLK)], in_=att_stg)

                ctx_n = small.tile([h, d], F32, tag="ctx_n")
                nc.scalar.activation(
                    out=ctx_n,
                    in_=ctx_ps,
                    func=mybir.ActivationFunctionType.Copy,
                    scale=recip_sb[:, ts(b, 1)],
                )
                for kc in range(CH):
                    tp = ps_t.tile([P, NSUB, 16], F32, tag="tps")
                    nc.tensor.transpose(
                        tp[:, 0, :h], ctx_n[:, ts(kc, P)], ident[:h, :h]
                    )
                    nc.any.tensor_copy(out=ctxT_sb[:, kc, b, :], in_=tp[:, 0, :h])

            # ---------------- output projection -------------------------
            # vh_all[A, b*16+h] = sum_c Wv[c, A] * ctxT[c, b, h]
            wv_sb = wpool.tile([P, CH, d], F32, tag="w")
            nc.sync.dma_start(
                out=wv_sb, in_=wv_d[:].rearrange("(o p) a -> p o a", p=P)
            )
            vh_sel = persist.tile([P, CH, bpc], F32, tag="vh_sel")
            for ka in range(CH):
                vh_ps = ps_epi.tile([P, BH], F32, tag="epi")
                for kc in range(CH):
                    nc.tensor.matmul(
                        vh_ps,
                        wv_sb[:, kc, ts(ka, P)],
                        ctxT_sb[:, kc, :, :],
                        start=(kc == 0),
                        stop=(kc == CH - 1),
                    )
                # gather diagonal: lanes a<64 -> head 2*ka, a>=64 -> 2*ka+1
                for half in range(HPC):
                    hh = HPC * ka + half
                    src = vh_ps[ts(half, 64), :].rearrange(
                        "p (b g) -> p b g", g=h
                    )[:, :, hh]
                    nc.any.tensor_copy(out=vh_sel[ts(half, 64), ka, :], in_=src)
            nc.vector.tensor_tensor(
                vh_sel,
                vh_sel,
                bv_sb[:, :, None].to_broadcast([P, CH, bpc]),
                mybir.AluOpType.add,
            )

            # out[b, j] = sum_A vh_sel[A, b] * Wo[A, j] + bo[j]
            wo_sb = wpool.tile([P, CH, d], F32, tag="w")
            nc.sync.dma_start(
                out=wo_sb, in_=wo_d[:].rearrange("(o p) j -> p o j", p=P)
            )
            out_ps = ps_epi.tile([bpc, d], F32, tag="epi")
            for ka in range(CH):
                for nh in range(2):
                    nc.tensor.matmul(
                        out_ps[:, ts(nh, 512)],
                        vh_sel[:, ka, :],
                        wo_sb[:, ka, ts(nh, 512)],
                        start=(ka == 0),
                        stop=(ka == CH - 1),
                    )
            out_sb = scratch.tile([bpc, d], F32, tag="out_sb")
            nc.vector.tensor_tensor(out_sb, out_ps, bo_rep, mybir.AluOpType.add)
            nc.sync.dma_start(out=out_d[:], in_=out_sb)

    nc.compile()
    return nc


def make_in_maps(vector, matrix, mask, Wq, bq, Wk, bk, Wv, bv, Wo, bo, bpc=BPC):
    """Host-side layout prep + sharding (no model FLOPs here)."""
    f = np.float32
    vecT = np.ascontiguousarray(np.asarray(vector, f).T)          # [D, B]
    wkT = np.ascontiguousarray(np.asarray(Wk, f).T)               # [A, D]
    mneg = np.where(np.asarray(mask) > 0, f(0.0), NEG).astype(f)  # [B, T]
    matrix = np.asarray(matrix, f)
    shared = {
        "Wq": np.ascontiguousarray(np.asarray(Wq, f)),
        "WkT": wkT,
        "Wv": np.ascontiguousarray(np.asarray(Wv, f)),
        "Wo": np.ascontiguousarray(np.asarray(Wo, f)),
        "bq": np.ascontiguousarray(np.asarray(bq, f)),
        "bk": np.ascontiguousarray(np.asarray(bk, f)),
        "bv": np.ascontiguousarray(np.asarray(bv, f)),
        "bo": np.ascontiguousarray(np.asarray(bo, f)),
    }
    n_cores = matrix.shape[0] // bpc
    in_maps = []
    for i in range(n_cores):
        sl = slice(i * bpc, (i + 1) * bpc)
        mN = np.ascontiguousarray(matrix[sl])
        in_maps.append(
            dict(
                shared,
                mN=mN,
                mT=np.ascontiguousarray(mN.transpose(0, 2, 1)),
                vecT=np.ascontiguousarray(vecT[:, sl]),
                mneg=np.ascontiguousarray(mneg[sl]),
            )
        )
    return in_maps


_CACHE = {}


def _get_nc(use_f32r):
    key = ("full", use_f32r)
    if key not in _CACHE:
        _CACHE[key] = build(use_f32r=use_f32r)
    return _CACHE[key]


def kernel(vector, matrix, mask, Wq, bq, Wk, bk, Wv, bv, Wo, bo):
    use_f32r = os.environ.get("MHA_F32R", "0") == "1"
    nc = _get_nc(use_f32r)
    in_maps = make_in_maps(vector, matrix, mask, Wq, bq, Wk, bk, Wv, bv, Wo, bo)
    res = run_bass_kernel_spmd(nc, in_maps, core_ids=list(range(NCORES)))
    out = np.concatenate([r["out_o"] for r in res.results], axis=0)
    attn = np.concatenate([r["attn_o"] for r in res.results], axis=0)
    scores = np.concatenate([r["scores_o"] for r in res.results], axis=0)
    kernel.last_exec_time_ns = res.exec_time_ns
    kernel.last_results = res
    return out, attn, scores


kernel.last_exec_time_ns = None
kernel.last_results = None
